# revision 3
# baseline (speedup 1.0000x reference)
"""GCN (2-layer graph convolution) on 8 TRN2 NeuronCores.

Strategy (1D graph partition):
  - Nodes sharded across 8 cores (12500 rows each); edges partitioned by
    destination row so segment_sum is core-local.
  - Layer 1: each core computes support1 = x_k @ W1 (bf16), AllGather ->
    full table T1 [100000, 128] bf16.
  - SpMM via dma_gather (4 SWDGE queues) of 256B rows + selection-matrix
    matmul segment-sum: per 128-edge chunk, S[e,d] = val[e]*(row[e]==d)
    built in ONE fused DVE tensor_scalar(is_equal, mult), then PE matmul
    accumulates into PSUM.
  - h^T = Relu(psum + b1) on ACT (bias along partitions), support2 = h@W2
    one matmul per tile, AllGather -> T2 [100000, 128] bf16 (cols 0:32 used).
  - SpMM2 same way (rhs width 32), then +b2 and log_softmax epilogue per tile.
  - Edges bucketed by col into 4 segments of 25000 so indices fit int16.
"""

import sys

sys.path.insert(0, "/opt/trn_rl_repo")

import numpy as np
import ml_dtypes

import concourse.bass as bass
import concourse.tile as tile
from concourse import bacc, mybir
from concourse.bass_utils import run_bass_kernel_spmd
from concourse.library_config import mlp

N = 100000
E = 3200000
F_IN, F_HID, F_OUT = 512, 128, 32
NC = 8
SHARD = N // NC          # 12500
P = 128
NT = (SHARD + P - 1) // P   # 98 tiles; last has 84 rows
NSEG = 4
SEG = N // NSEG          # 25000 (fits int16 indices)
BF16 = ml_dtypes.bfloat16


def _preprocess(edge_row, edge_col, edge_val):
    """Sort/pad edges into per-(core, dst-tile, col-segment) buckets of
    whole 128-edge chunks, identical chunk counts across cores."""
    er = edge_row.astype(np.int64)
    ec = edge_col.astype(np.int64)
    k = er // SHARD
    t = (er % SHARD) // P
    s = ec // SEG
    key = (k * NT + t) * NSEG + s
    order = np.argsort(key, kind="stable")
    counts = np.bincount(key, minlength=NC * NT * NSEG).reshape(NC, NT, NSEG)
    C_ts = -(-counts.max(axis=0) // P)          # [NT, NSEG] chunks per bucket
    TC = int(C_ts.sum())
    off_ts = np.zeros((NT, NSEG), np.int64)
    off_flat = np.concatenate([[0], np.cumsum(C_ts.flatten())])[:-1]
    off_ts[:] = off_flat.reshape(NT, NSEG)

    starts = np.zeros(NC * NT * NSEG + 1, np.int64)
    starts[1:] = np.cumsum(counts.flatten())
    key_s = key[order]
    rank = np.arange(E, dtype=np.int64) - starts[key_s]
    slot_base = np.zeros(NC * NT * NSEG, np.int64)
    base_kts = (np.arange(NC)[:, None, None] * TC + off_ts[None]) * P
    slot_base[:] = base_kts.reshape(-1)
    slot = slot_base[key_s] + rank

    idx_slots = np.zeros(NC * TC * P, np.int16)
    rows_slots = np.zeros(NC * TC * P, np.float32)
    vals_slots = np.zeros(NC * TC * P, np.float32)
    ero, eco, evo = edge_row[order], edge_col[order], edge_val[order]
    idx_slots[slot] = (eco % SEG).astype(np.int16)
    rows_slots[slot] = ((ero % SHARD) % P).astype(np.float32)
    vals_slots[slot] = evo.astype(np.float32)

    idx_slots = idx_slots.reshape(NC, TC * P)
    idx16 = np.stack(
        [np.tile(idx_slots[c].reshape(-1, 16).T, (8, 1)) for c in range(NC)]
    )                                                     # [NC, 128, 8*TC]
    rows16 = np.ascontiguousarray(
        rows_slots.reshape(NC, TC, P).transpose(0, 2, 1))  # [NC, 128, TC]
    vals16 = np.ascontiguousarray(
        vals_slots.reshape(NC, TC, P).transpose(0, 2, 1))  # [NC, 128, TC]
    return C_ts, off_ts, TC, idx16, rows16, vals16


def _build_program(C_ts, off_ts, TC):
    f32, bf16, i16 = mybir.dt.float32, mybir.dt.bfloat16, mybir.dt.int16
    nc = bacc.Bacc("TRN2", target_bir_lowering=False, debug=False,
                   num_devices=NC, num_swdge_queues=4)

    xT = nc.dram_tensor("xT", [F_IN, SHARD], bf16, kind="ExternalInput")
    W1b = nc.dram_tensor("W1b", [F_IN, F_HID], bf16, kind="ExternalInput")
    W2b = nc.dram_tensor("W2b", [F_HID, F_OUT], bf16, kind="ExternalInput")
    b1c = nc.dram_tensor("b1c", [P, 1], f32, kind="ExternalInput")
    b2bc = nc.dram_tensor("b2bc", [P, F_OUT], f32, kind="ExternalInput")
    iota = nc.dram_tensor("iota", [P, P], f32, kind="ExternalInput")
    idx16 = nc.dram_tensor("idx16", [P, 8 * TC], i16, kind="ExternalInput")
    rowsl = nc.dram_tensor("rowsl", [P, TC], f32, kind="ExternalInput")
    valsl = nc.dram_tensor("valsl", [P, TC], f32, kind="ExternalInput")
    out = nc.dram_tensor("out", [SHARD, F_OUT], f32, kind="ExternalOutput")

    T1_local = nc.dram_tensor("T1_local", [SHARD, F_HID], bf16)
    T1_full = nc.dram_tensor("T1_full", [N, F_HID], bf16, addr_space="Shared")
    T2_local = nc.dram_tensor("T2_local", [SHARD, P], bf16)
    T2_full = nc.dram_tensor("T2_full", [N, P], bf16, addr_space="Shared")

    C_t = C_ts.sum(axis=1)          # chunks per tile
    CMAX = int(C_ts.max())

    with tile.TileContext(nc) as tc:
        with (
            tc.tile_pool(name="const", bufs=1) as cpool,
            tc.tile_pool(name="xa", bufs=3) as xapool,
            tc.tile_pool(name="s1o", bufs=3) as s1pool,
            tc.tile_pool(name="g", bufs=8) as gpool,
            tc.tile_pool(name="sm", bufs=6) as smpool,
            tc.tile_pool(name="meta", bufs=3) as mpool,
            tc.tile_pool(name="ep", bufs=4) as eppool,
            tc.tile_pool(name="pa", bufs=2, space="PSUM") as papool,
            tc.tile_pool(name="ph", bufs=2, space="PSUM") as phpool,
            tc.tile_pool(name="ps", bufs=2, space="PSUM") as pspool,
        ):
            nc.gpsimd.load_library(mlp)

            # ---- resident constants ----
            W1_sb = cpool.tile([P, 4, F_HID], bf16)
            nc.sync.dma_start(
                W1_sb[:], W1b.ap().rearrange("(kk p) f -> p kk f", p=P))
            W2_sb = cpool.tile([P, F_OUT], bf16)
            nc.sync.dma_start(W2_sb[:], W2b.ap())
            b1_sb = cpool.tile([P, 1], f32)
            nc.sync.dma_start(b1_sb[:], b1c.ap())
            b2_sb = cpool.tile([P, F_OUT], f32)
            nc.sync.dma_start(b2_sb[:], b2bc.ap())
            iota_sb = cpool.tile([P, P], f32)
            nc.sync.dma_start(iota_sb[:], iota.ap())

            # ---- phase A: support1 = x_k @ W1 -> T1_local (bf16) ----
            for m in range(NT):
                m0 = m * P
                mw = min(P, SHARD - m0)
                xa = xapool.tile([P, 4, P], bf16)
                nc.sync.dma_start(
                    xa[:, :, :mw],
                    xT.ap()[:, m0:m0 + mw].rearrange("(kk p) m -> p kk m", p=P))
                ps = papool.tile([P, F_HID], f32, space="PSUM")
                for kk in range(4):
                    nc.tensor.matmul(ps[:mw, :], xa[:, kk, :mw], W1_sb[:, kk, :],
                                     start=(kk == 0), stop=(kk == 3))
                s1 = s1pool.tile([P, F_HID], bf16)
                nc.scalar.activation(s1[:mw, :], ps[:mw, :],
                                     mybir.ActivationFunctionType.Copy)
                nc.sync.dma_start(T1_local.ap()[m0:m0 + mw, :], s1[:mw, :])

            # ---- AllGather T1 ----
            nc.gpsimd.collective_compute(
                "AllGather", mybir.AluOpType.bypass,
                replica_groups=[list(range(NC))],
                ins=[T1_local.ap().opt()],
                outs=[T1_full.ap().opt()],
            )

            # ---- phase B: SpMM1 + Relu + @W2 -> T2_local ----
            for t in range(NT):
                t0 = t * P
                tw = min(P, SHARD - t0)
                ct = int(C_t[t])
                coff = int(off_ts[t, 0])
                rv = mpool.tile([P, 2 * CMAX * NSEG], f32, tag="rv")
                nc.sync.dma_start(rv[:, :ct], rowsl.ap()[:, coff:coff + ct])
                nc.sync.dma_start(rv[:, ct:2 * ct], valsl.ap()[:, coff:coff + ct])
                ix = mpool.tile([P, 8 * CMAX * NSEG], i16, tag="ix")
                nc.sync.dma_start(ix[:, :8 * ct],
                                  idx16.ap()[:, 8 * coff:8 * (coff + ct)])

                ph = phpool.tile([P, P], f32, space="PSUM")
                ci = 0
                for s in range(NSEG):
                    cs = int(C_ts[t, s])
                    if cs == 0:
                        continue
                    local_off = int(off_ts[t, s]) - coff
                    g = gpool.tile([P, CMAX, P], bf16, tag="g")
                    nc.gpsimd.dma_gather(
                        g[:, :cs, :],
                        T1_full.ap()[s * SEG:(s + 1) * SEG, :],
                        ix[:, 8 * local_off:8 * (local_off + cs)],
                        cs * P, cs * P, F_HID,
                        single_packet=False, queue_num=s,
                    )
                    for c in range(cs):
                        sm = smpool.tile([P, P], bf16, tag="sm")
                        nc.vector.tensor_scalar(
                            sm[:], iota_sb[:],
                            rv[:, local_off + c:local_off + c + 1],
                            rv[:, ct + local_off + c:ct + local_off + c + 1],
                            op0=mybir.AluOpType.is_equal,
                            op1=mybir.AluOpType.mult)
                        nc.tensor.matmul(ph[:], g[:, c, :], sm[:],
                                         start=(ci == 0), stop=(ci == ct - 1))
                        ci += 1
                # h^T = relu(ph + b1) ; support2 = h @ W2
                hT = eppool.tile([P, P], bf16, tag="hT")
                nc.scalar.activation(hT[:], ph[:],
                                     mybir.ActivationFunctionType.Relu,
                                     bias=b1_sb[:])
                ps2 = pspool.tile([P, F_OUT], f32, space="PSUM")
                nc.tensor.matmul(ps2[:], hT[:], W2_sb[:], start=True, stop=True)
                s2 = eppool.tile([P, F_OUT], bf16, tag="s2")
                nc.scalar.activation(s2[:], ps2[:],
                                     mybir.ActivationFunctionType.Copy)
                nc.sync.dma_start(T2_local.ap()[t0:t0 + tw, :F_OUT], s2[:tw, :])

            # ---- AllGather T2 ----
            nc.gpsimd.collective_compute(
                "AllGather", mybir.AluOpType.bypass,
                replica_groups=[list(range(NC))],
                ins=[T2_local.ap().opt()],
                outs=[T2_full.ap().opt()],
            )

            # ---- phase D: SpMM2 + bias + log_softmax -> out ----
            for t in range(NT):
                t0 = t * P
                tw = min(P, SHARD - t0)
                ct = int(C_t[t])
                coff = int(off_ts[t, 0])
                rv = mpool.tile([P, 2 * CMAX * NSEG], f32, tag="rv")
                nc.sync.dma_start(rv[:, :ct], rowsl.ap()[:, coff:coff + ct])
                nc.sync.dma_start(rv[:, ct:2 * ct], valsl.ap()[:, coff:coff + ct])
                ix = mpool.tile([P, 8 * CMAX * NSEG], i16, tag="ix")
                nc.sync.dma_start(ix[:, :8 * ct],
                                  idx16.ap()[:, 8 * coff:8 * (coff + ct)])

                pl = pspool.tile([P, F_OUT], f32, space="PSUM", tag="pl")
                ci = 0
                for s in range(NSEG):
                    cs = int(C_ts[t, s])
                    if cs == 0:
                        continue
                    local_off = int(off_ts[t, s]) - coff
                    g = gpool.tile([P, CMAX, P], bf16, tag="g")
                    nc.gpsimd.dma_gather(
                        g[:, :cs, :],
                        T2_full.ap()[s * SEG:(s + 1) * SEG, :],
                        ix[:, 8 * local_off:8 * (local_off + cs)],
                        cs * P, cs * P, P,
                        single_packet=False, queue_num=s,
                    )
                    for c in range(cs):
                        sm = smpool.tile([P, P], bf16, tag="sm")
                        nc.vector.tensor_scalar(
                            sm[:], iota_sb[:],
                            rv[:, local_off + c:local_off + c + 1],
                            rv[:, ct + local_off + c:ct + local_off + c + 1],
                            op0=mybir.AluOpType.is_equal,
                            op1=mybir.AluOpType.mult)
                        nc.tensor.matmul(pl[:], sm[:], g[:, c, :F_OUT],
                                         start=(ci == 0), stop=(ci == ct - 1))
                        ci += 1
                # logits = pl + b2 ; out = log_softmax(logits)
                lg = eppool.tile([P, F_OUT], f32, tag="lg")
                nc.vector.tensor_add(lg[:], pl[:], b2_sb[:])
                mx = eppool.tile([P, 1], f32, tag="mx")
                nc.vector.reduce_max(mx[:], lg[:], axis=mybir.AxisListType.X)
                nmx = eppool.tile([P, 1], f32, tag="nmx")
                nc.vector.tensor_scalar_mul(nmx[:], mx[:], -1.0)
                ex = eppool.tile([P, F_OUT], f32, tag="ex")
                nc.scalar.activation(ex[:], lg[:],
                                     mybir.ActivationFunctionType.Exp,
                                     bias=nmx[:])
                sme = eppool.tile([P, 1], f32, tag="sme")
                nc.vector.reduce_sum(sme[:], ex[:], axis=mybir.AxisListType.X)
                lns = eppool.tile([P, 1], f32, tag="lns")
                nc.scalar.activation(lns[:], sme[:],
                                     mybir.ActivationFunctionType.Ln)
                mls = eppool.tile([P, 1], f32, tag="mls")
                nc.vector.tensor_add(mls[:], mx[:], lns[:])
                oo = eppool.tile([P, F_OUT], f32, tag="oo")
                nc.vector.tensor_scalar(
                    oo[:], lg[:], mls[:], None,
                    op0=mybir.AluOpType.subtract)
                nc.sync.dma_start(out.ap()[t0:t0 + tw, :], oo[:tw, :])

    nc.compile()
    return nc


def _prepare(x, edge_row, edge_col, edge_val, W1, b1, W2, b2):
    C_ts, off_ts, TC, idx16, rows16, vals16 = _preprocess(
        np.asarray(edge_row), np.asarray(edge_col), np.asarray(edge_val))
    nc = _build_program(C_ts, off_ts, TC)

    x = np.asarray(x, np.float32)
    W1 = np.asarray(W1, np.float32)
    W2 = np.asarray(W2, np.float32)
    b1 = np.asarray(b1, np.float32)
    b2 = np.asarray(b2, np.float32)

    iota_np = np.broadcast_to(
        np.arange(P, dtype=np.float32)[None, :], (P, P)).copy()
    b1_np = b1.reshape(F_HID, 1).astype(np.float32)
    b2_np = np.broadcast_to(b2[None, :], (P, F_OUT)).copy().astype(np.float32)
    W1_np = W1.astype(BF16)
    W2_np = W2.astype(BF16)

    in_maps = []
    for c in range(NC):
        xk = x[c * SHARD:(c + 1) * SHARD]
        in_maps.append({
            "xT": np.ascontiguousarray(xk.T).astype(BF16),
            "W1b": W1_np, "W2b": W2_np,
            "b1c": b1_np, "b2bc": b2_np, "iota": iota_np,
            "idx16": idx16[c], "rowsl": rows16[c], "valsl": vals16[c],
        })

    return nc, in_maps


def kernel(x, edge_row, edge_col, edge_val, W1, b1, W2, b2):
    nc, in_maps = _prepare(x, edge_row, edge_col, edge_val, W1, b1, W2, b2)
    res = run_bass_kernel_spmd(nc, in_maps, core_ids=list(range(NC)),
                               trace=False)
    return np.concatenate([res.results[c]["out"] for c in range(NC)], axis=0)


# revision 8
# speedup vs baseline: 1.3410x; 1.3410x over previous
"""GCN (2-layer graph convolution) on 8 TRN2 NeuronCores.

Strategy (1D graph partition):
  - Nodes sharded across 8 cores (12500 rows each); edges partitioned by
    destination row so segment_sum is core-local.
  - Layer 1: each core computes support1 = x_k @ W1 (bf16), AllGather ->
    full table T1 [100000, 128] bf16.
  - SpMM via dma_gather (4 SWDGE queues) of 256B rows + selection-matrix
    matmul segment-sum: per 128-edge chunk, S[e,d] = val[e]*(row[e]==d)
    built in ONE fused DVE tensor_scalar(is_equal, mult), then PE matmul
    accumulates into PSUM.
  - h^T = Relu(psum + b1) on ACT (bias along partitions), support2 = h@W2
    one matmul per tile, AllGather -> T2 [100000, 128] bf16 (cols 0:32 used).
  - SpMM2 same way (rhs width 32), then +b2 and log_softmax epilogue per tile.
  - Edges bucketed by col into 4 segments of 25000 so indices fit int16.
"""

import sys

sys.path.insert(0, "/opt/trn_rl_repo")

import numpy as np
import ml_dtypes

import concourse.bass as bass
import concourse.tile as tile
from concourse import bacc, mybir
from concourse.bass_utils import run_bass_kernel_spmd
from concourse.library_config import mlp

N = 100000
E = 3200000
F_IN, F_HID, F_OUT = 512, 128, 32
NC = 8
SHARD = N // NC          # 12500
P = 128
NT = (SHARD + P - 1) // P   # 98 tiles; last has 84 rows
NSEG = 4
SEG = N // NSEG          # 25000 (fits int16 indices)
BF16 = ml_dtypes.bfloat16


def _preprocess(edge_row, edge_col, edge_val):
    """Sort/pad edges into per-(core, dst-tile, col-segment) buckets of
    whole 128-edge chunks, identical chunk counts across cores."""
    er = edge_row.astype(np.int64)
    ec = edge_col.astype(np.int64)
    k = er // SHARD
    t = (er % SHARD) // P
    s = ec // SEG
    key = (k * NT + t) * NSEG + s
    order = np.argsort(key, kind="stable")
    counts = np.bincount(key, minlength=NC * NT * NSEG).reshape(NC, NT, NSEG)
    C_ts = -(-counts.max(axis=0) // P)          # [NT, NSEG] chunks per bucket
    TC = int(C_ts.sum())
    off_ts = np.zeros((NT, NSEG), np.int64)
    off_flat = np.concatenate([[0], np.cumsum(C_ts.flatten())])[:-1]
    off_ts[:] = off_flat.reshape(NT, NSEG)

    starts = np.zeros(NC * NT * NSEG + 1, np.int64)
    starts[1:] = np.cumsum(counts.flatten())
    key_s = key[order]
    rank = np.arange(E, dtype=np.int64) - starts[key_s]
    slot_base = np.zeros(NC * NT * NSEG, np.int64)
    base_kts = (np.arange(NC)[:, None, None] * TC + off_ts[None]) * P
    slot_base[:] = base_kts.reshape(-1)
    slot = slot_base[key_s] + rank

    idx_slots = np.zeros(NC * TC * P, np.int16)
    rows_slots = np.zeros(NC * TC * P, np.float32)
    vals_slots = np.zeros(NC * TC * P, np.float32)
    ero, eco, evo = edge_row[order], edge_col[order], edge_val[order]
    idx_slots[slot] = (eco % SEG).astype(np.int16)
    rows_slots[slot] = ((ero % SHARD) % P).astype(np.float32)
    vals_slots[slot] = evo.astype(np.float32)

    idx_slots = idx_slots.reshape(NC, TC * P)
    idx16 = np.stack(
        [np.tile(idx_slots[c].reshape(-1, 16).T, (8, 1)) for c in range(NC)]
    )                                                     # [NC, 128, 8*TC]
    rows16 = np.ascontiguousarray(
        rows_slots.reshape(NC, TC, P).transpose(0, 2, 1))  # [NC, 128, TC]
    vals16 = np.ascontiguousarray(
        vals_slots.reshape(NC, TC, P).transpose(0, 2, 1))  # [NC, 128, TC]
    return C_ts, off_ts, TC, idx16, rows16, vals16


def _build_program(C_ts, off_ts, TC):
    f32, bf16, i16 = mybir.dt.float32, mybir.dt.bfloat16, mybir.dt.int16
    nc = bacc.Bacc("TRN2", target_bir_lowering=False, debug=False,
                   num_devices=NC, num_swdge_queues=4)

    xT = nc.dram_tensor("xT", [F_IN, SHARD], bf16, kind="ExternalInput")
    W1b = nc.dram_tensor("W1b", [F_IN, F_HID], bf16, kind="ExternalInput")
    W2b = nc.dram_tensor("W2b", [F_HID, F_OUT], bf16, kind="ExternalInput")
    b1c = nc.dram_tensor("b1c", [P, 1], f32, kind="ExternalInput")
    b2bc = nc.dram_tensor("b2bc", [P, F_OUT], f32, kind="ExternalInput")
    # odd free dim (129) + bf16 keeps the S-build tensor_scalar in single-port
    # 2x_1P DVE mode -- 2-port modes lock GPSIMD out of the shared SBUF port
    # and stall dma_gather descriptor generation.
    iota = nc.dram_tensor("iota", [P, P + 1], bf16, kind="ExternalInput")
    idx16 = nc.dram_tensor("idx16", [P, 8 * TC], i16, kind="ExternalInput")
    rowsl = nc.dram_tensor("rowsl", [P, TC], f32, kind="ExternalInput")
    valsl = nc.dram_tensor("valsl", [P, TC], f32, kind="ExternalInput")
    out = nc.dram_tensor("out", [SHARD, F_OUT], f32, kind="ExternalOutput")

    T1_local = nc.dram_tensor("T1_local", [SHARD, F_HID], bf16)
    T1_full = nc.dram_tensor("T1_full", [N, F_HID], bf16, addr_space="Shared")
    T2_local = nc.dram_tensor("T2_local", [SHARD, P], bf16)
    T2_full = nc.dram_tensor("T2_full", [N, P], bf16, addr_space="Shared")

    C_t = C_ts.sum(axis=1)          # chunks per tile
    CMAX = int(C_ts.max())

    with tile.TileContext(nc) as tc:
        with (
            tc.tile_pool(name="const", bufs=1) as cpool,
            tc.tile_pool(name="xa", bufs=3) as xapool,
            tc.tile_pool(name="s1o", bufs=3) as s1pool,
            tc.tile_pool(name="g", bufs=8) as gpool,
            tc.tile_pool(name="sm", bufs=6) as smpool,
            tc.tile_pool(name="meta", bufs=3) as mpool,
            tc.tile_pool(name="ep", bufs=4) as eppool,
            tc.tile_pool(name="pa", bufs=2, space="PSUM") as papool,
            tc.tile_pool(name="ph", bufs=2, space="PSUM") as phpool,
            tc.tile_pool(name="ps", bufs=2, space="PSUM") as pspool,
        ):
            nc.gpsimd.load_library(mlp)

            # ---- resident constants ----
            W1_sb = cpool.tile([P, 4, F_HID], bf16)
            nc.sync.dma_start(
                W1_sb[:], W1b.ap().rearrange("(kk p) f -> p kk f", p=P))
            W2_sb = cpool.tile([P, F_OUT], bf16)
            nc.sync.dma_start(W2_sb[:], W2b.ap())
            b1_sb = cpool.tile([P, 1], f32)
            nc.sync.dma_start(b1_sb[:], b1c.ap())
            b2_sb = cpool.tile([P, F_OUT], f32)
            nc.sync.dma_start(b2_sb[:], b2bc.ap())
            iota_sb = cpool.tile([P, P + 1], bf16)
            nc.sync.dma_start(iota_sb[:], iota.ap())

            # ---- phase A: support1 = x_k @ W1 -> T1_local (bf16) ----
            for m in range(NT):
                m0 = m * P
                mw = min(P, SHARD - m0)
                xa = xapool.tile([P, 4, P], bf16)
                nc.sync.dma_start(
                    xa[:, :, :mw],
                    xT.ap()[:, m0:m0 + mw].rearrange("(kk p) m -> p kk m", p=P))
                ps = papool.tile([P, F_HID], f32, space="PSUM")
                for kk in range(4):
                    nc.tensor.matmul(ps[:mw, :], xa[:, kk, :mw], W1_sb[:, kk, :],
                                     start=(kk == 0), stop=(kk == 3))
                s1 = s1pool.tile([P, F_HID], bf16)
                nc.scalar.activation(s1[:mw, :], ps[:mw, :],
                                     mybir.ActivationFunctionType.Copy)
                nc.sync.dma_start(T1_local.ap()[m0:m0 + mw, :], s1[:mw, :])

            # ---- AllGather T1 ----
            nc.gpsimd.collective_compute(
                "AllGather", mybir.AluOpType.bypass,
                replica_groups=[list(range(NC))],
                ins=[T1_local.ap().opt()],
                outs=[T1_full.ap().opt()],
            )

            # ---- phase B: SpMM1 + Relu + @W2 -> T2_local ----
            for t in range(NT):
                t0 = t * P
                tw = min(P, SHARD - t0)
                ct = int(C_t[t])
                coff = int(off_ts[t, 0])
                rv = mpool.tile([P, 2 * CMAX * NSEG], f32, tag="rv")
                nc.sync.dma_start(rv[:, :ct], rowsl.ap()[:, coff:coff + ct])
                nc.sync.dma_start(rv[:, ct:2 * ct], valsl.ap()[:, coff:coff + ct])
                ix = mpool.tile([P, 8 * CMAX * NSEG], i16, tag="ix")
                nc.sync.dma_start(ix[:, :8 * ct],
                                  idx16.ap()[:, 8 * coff:8 * (coff + ct)])

                ph = phpool.tile([P, P], f32, space="PSUM")
                ci = 0
                for s in range(NSEG):
                    cs = int(C_ts[t, s])
                    if cs == 0:
                        continue
                    local_off = int(off_ts[t, s]) - coff
                    g = gpool.tile([P, CMAX, P], bf16, tag="g")
                    nc.gpsimd.dma_gather(
                        g[:, :cs, :],
                        T1_full.ap()[s * SEG:(s + 1) * SEG, :],
                        ix[:, 8 * local_off:8 * (local_off + cs)],
                        cs * P, cs * P, F_HID,
                        single_packet=False, queue_num=s,
                    )
                    for c in range(cs):
                        sm = smpool.tile([P, P + 1], bf16, tag="sm")
                        nc.vector.tensor_scalar(
                            sm[:], iota_sb[:],
                            rv[:, local_off + c:local_off + c + 1],
                            rv[:, ct + local_off + c:ct + local_off + c + 1],
                            op0=mybir.AluOpType.is_equal,
                            op1=mybir.AluOpType.mult)
                        nc.tensor.matmul(ph[:], g[:, c, :], sm[:, :P],
                                         start=(ci == 0), stop=(ci == ct - 1))
                        ci += 1
                # h^T = relu(ph + b1) ; support2 = h @ W2
                hT = eppool.tile([P, P], bf16, tag="hT")
                nc.scalar.activation(hT[:], ph[:],
                                     mybir.ActivationFunctionType.Relu,
                                     bias=b1_sb[:])
                ps2 = pspool.tile([P, F_OUT], f32, space="PSUM")
                nc.tensor.matmul(ps2[:], hT[:], W2_sb[:], start=True, stop=True)
                s2 = eppool.tile([P, F_OUT], bf16, tag="s2")
                nc.vector.tensor_copy(s2[:], ps2[:])
                nc.sync.dma_start(T2_local.ap()[t0:t0 + tw, :F_OUT], s2[:tw, :])

            # ---- AllGather T2 ----
            nc.gpsimd.collective_compute(
                "AllGather", mybir.AluOpType.bypass,
                replica_groups=[list(range(NC))],
                ins=[T2_local.ap().opt()],
                outs=[T2_full.ap().opt()],
            )

            # ---- phase D: SpMM2 + bias + log_softmax -> out ----
            for t in range(NT):
                t0 = t * P
                tw = min(P, SHARD - t0)
                ct = int(C_t[t])
                coff = int(off_ts[t, 0])
                rv = mpool.tile([P, 2 * CMAX * NSEG], f32, tag="rv")
                nc.sync.dma_start(rv[:, :ct], rowsl.ap()[:, coff:coff + ct])
                nc.sync.dma_start(rv[:, ct:2 * ct], valsl.ap()[:, coff:coff + ct])
                ix = mpool.tile([P, 8 * CMAX * NSEG], i16, tag="ix")
                nc.sync.dma_start(ix[:, :8 * ct],
                                  idx16.ap()[:, 8 * coff:8 * (coff + ct)])

                pl = pspool.tile([P, F_OUT], f32, space="PSUM", tag="pl")
                ci = 0
                for s in range(NSEG):
                    cs = int(C_ts[t, s])
                    if cs == 0:
                        continue
                    local_off = int(off_ts[t, s]) - coff
                    g = gpool.tile([P, CMAX, P], bf16, tag="g")
                    nc.gpsimd.dma_gather(
                        g[:, :cs, :],
                        T2_full.ap()[s * SEG:(s + 1) * SEG, :],
                        ix[:, 8 * local_off:8 * (local_off + cs)],
                        cs * P, cs * P, P,
                        single_packet=False, queue_num=s,
                    )
                    for c in range(cs):
                        sm = smpool.tile([P, P + 1], bf16, tag="sm")
                        nc.vector.tensor_scalar(
                            sm[:], iota_sb[:],
                            rv[:, local_off + c:local_off + c + 1],
                            rv[:, ct + local_off + c:ct + local_off + c + 1],
                            op0=mybir.AluOpType.is_equal,
                            op1=mybir.AluOpType.mult)
                        nc.tensor.matmul(pl[:], sm[:, :P], g[:, c, :F_OUT],
                                         start=(ci == 0), stop=(ci == ct - 1))
                        ci += 1
                # logits = pl + b2 ; out = log_softmax(logits)
                lg = eppool.tile([P, F_OUT], f32, tag="lg")
                nc.vector.tensor_add(lg[:], pl[:], b2_sb[:])
                mx = eppool.tile([P, 1], f32, tag="mx")
                nc.vector.reduce_max(mx[:], lg[:], axis=mybir.AxisListType.X)
                nmx = eppool.tile([P, 1], f32, tag="nmx")
                nc.vector.tensor_scalar_mul(nmx[:], mx[:], -1.0)
                ex = eppool.tile([P, F_OUT], f32, tag="ex")
                nc.scalar.activation(ex[:], lg[:],
                                     mybir.ActivationFunctionType.Exp,
                                     bias=nmx[:])
                sme = eppool.tile([P, 1], f32, tag="sme")
                nc.vector.reduce_sum(sme[:], ex[:], axis=mybir.AxisListType.X)
                lns = eppool.tile([P, 1], f32, tag="lns")
                nc.scalar.activation(lns[:], sme[:],
                                     mybir.ActivationFunctionType.Ln)
                mls = eppool.tile([P, 1], f32, tag="mls")
                nc.vector.tensor_add(mls[:], mx[:], lns[:])
                oo = eppool.tile([P, F_OUT], f32, tag="oo")
                nc.vector.tensor_scalar(
                    oo[:], lg[:], mls[:], None,
                    op0=mybir.AluOpType.subtract)
                nc.sync.dma_start(out.ap()[t0:t0 + tw, :], oo[:tw, :])

    nc.compile()
    return nc


def _prepare(x, edge_row, edge_col, edge_val, W1, b1, W2, b2):
    C_ts, off_ts, TC, idx16, rows16, vals16 = _preprocess(
        np.asarray(edge_row), np.asarray(edge_col), np.asarray(edge_val))
    nc = _build_program(C_ts, off_ts, TC)

    x = np.asarray(x, np.float32)
    W1 = np.asarray(W1, np.float32)
    W2 = np.asarray(W2, np.float32)
    b1 = np.asarray(b1, np.float32)
    b2 = np.asarray(b2, np.float32)

    iota_np = np.broadcast_to(
        np.arange(P + 1, dtype=np.float32)[None, :], (P, P + 1)).astype(BF16)
    b1_np = b1.reshape(F_HID, 1).astype(np.float32)
    b2_np = np.broadcast_to(b2[None, :], (P, F_OUT)).copy().astype(np.float32)
    W1_np = W1.astype(BF16)
    W2_np = W2.astype(BF16)

    in_maps = []
    for c in range(NC):
        xk = x[c * SHARD:(c + 1) * SHARD]
        in_maps.append({
            "xT": np.ascontiguousarray(xk.T).astype(BF16),
            "W1b": W1_np, "W2b": W2_np,
            "b1c": b1_np, "b2bc": b2_np, "iota": iota_np,
            "idx16": idx16[c], "rowsl": rows16[c], "valsl": vals16[c],
        })

    return nc, in_maps


def kernel(x, edge_row, edge_col, edge_val, W1, b1, W2, b2):
    nc, in_maps = _prepare(x, edge_row, edge_col, edge_val, W1, b1, W2, b2)
    res = run_bass_kernel_spmd(nc, in_maps, core_ids=list(range(NC)),
                               trace=False)
    return np.concatenate([res.results[c]["out"] for c in range(NC)], axis=0)


# revision 9
# speedup vs baseline: 1.9013x; 1.4179x over previous
"""GCN (2-layer graph convolution) on 8 TRN2 NeuronCores.

Strategy (1D graph partition):
  - Nodes sharded across 8 cores (12500 rows each); edges partitioned by
    destination row so segment_sum is core-local.
  - Layer 1: each core computes support1 = x_k @ W1 (bf16), AllGather ->
    full table T1 [100000, 128] bf16.
  - SpMM via dma_gather (4 SWDGE queues) of 256B rows + selection-matrix
    matmul segment-sum: per 128-edge chunk, S[e,d] = val[e]*(row[e]==d).
    S is precomputed on the host (it depends only on edge structure) and
    streamed in as sequential DMA, so no per-chunk vector-engine work.
  - h^T = Relu(psum + b1) on ACT (bias along partitions), support2 = h@W2
    one matmul per tile, AllGather -> T2 [100000, 128] bf16 (cols 0:32 used).
  - SpMM2 same way (rhs width 32), then +b2 and log_softmax epilogue.
  - Edges bucketed by col into 4 segments of 25000 so indices fit int16.
"""

import sys

sys.path.insert(0, "/opt/trn_rl_repo")

import numpy as np
import ml_dtypes

import concourse.bass as bass
import concourse.tile as tile
from concourse import bacc, mybir
from concourse.bass_utils import run_bass_kernel_spmd
from concourse.library_config import mlp

N = 100000
E = 3200000
F_IN, F_HID, F_OUT = 512, 128, 32
NC = 8
SHARD = N // NC          # 12500
P = 128
NT = (SHARD + P - 1) // P   # 98 tiles; last has 84 rows
NSEG = 4
SEG = N // NSEG          # 25000 (fits int16 indices)
BF16 = ml_dtypes.bfloat16


def _preprocess(edge_row, edge_col, edge_val):
    """Sort/pad edges into per-(core, dst-tile, col-segment) buckets of
    whole 128-edge chunks (chunk counts identical across cores), and build
    the per-chunk selection matrices S[e, d] = val[e] * (row_local[e] == d).
    """
    er = edge_row.astype(np.int64)
    ec = edge_col.astype(np.int64)
    k = er // SHARD
    t = (er % SHARD) // P
    s = ec // SEG
    key = (k * NT + t) * NSEG + s
    order = np.argsort(key, kind="stable")
    counts = np.bincount(key, minlength=NC * NT * NSEG).reshape(NC, NT, NSEG)
    C_ts = -(-counts.max(axis=0) // P)          # [NT, NSEG] chunks per bucket
    TC = int(C_ts.sum())
    off_flat = np.concatenate([[0], np.cumsum(C_ts.flatten())])[:-1]
    off_ts = off_flat.reshape(NT, NSEG)

    starts = np.zeros(NC * NT * NSEG + 1, np.int64)
    starts[1:] = np.cumsum(counts.flatten())
    key_s = key[order]
    rank = np.arange(E, dtype=np.int64) - starts[key_s]
    base_kts = (np.arange(NC)[:, None, None] * TC + off_ts[None]) * P
    slot = base_kts.reshape(-1)[key_s] + rank

    idx_slots = np.zeros(NC * TC * P, np.int16)
    rows_slots = np.zeros(NC * TC * P, np.int64)
    vals_slots = np.zeros(NC * TC * P, np.float32)
    ero, eco, evo = edge_row[order], edge_col[order], edge_val[order]
    idx_slots[slot] = (eco % SEG).astype(np.int16)
    rows_slots[slot] = (ero % SHARD) % P
    vals_slots[slot] = evo.astype(np.float32)

    idx_slots = idx_slots.reshape(NC, TC * P)
    idx16 = np.stack(
        [np.tile(idx_slots[c].reshape(-1, 16).T, (8, 1)) for c in range(NC)]
    )                                                     # [NC, 128, 8*TC]

    # S matrices: smat[core][p, ci*128 + d] = S_chunk_ci[p, d]
    smats = []
    rows_k = rows_slots.reshape(NC, TC * P)
    vals_k = vals_slots.reshape(NC, TC * P)
    for c in range(NC):
        S = np.zeros((TC * P, P), dtype=BF16)
        S[np.arange(TC * P), rows_k[c]] = vals_k[c].astype(BF16)
        smats.append(np.ascontiguousarray(
            S.reshape(TC, P, P).transpose(1, 0, 2).reshape(P, TC * P)))
    return C_ts, off_ts, TC, idx16, smats


def _build_program(C_ts, off_ts, TC):
    f32, bf16, i16 = mybir.dt.float32, mybir.dt.bfloat16, mybir.dt.int16
    nc = bacc.Bacc("TRN2", target_bir_lowering=False, debug=False,
                   num_devices=NC, num_swdge_queues=4)

    xT = nc.dram_tensor("xT", [F_IN, SHARD], bf16, kind="ExternalInput")
    W1b = nc.dram_tensor("W1b", [F_IN, F_HID], bf16, kind="ExternalInput")
    W2b = nc.dram_tensor("W2b", [F_HID, F_OUT], bf16, kind="ExternalInput")
    b1c = nc.dram_tensor("b1c", [P, 1], f32, kind="ExternalInput")
    b2bc = nc.dram_tensor("b2bc", [P, F_OUT], f32, kind="ExternalInput")
    idx16 = nc.dram_tensor("idx16", [P, 8 * TC], i16, kind="ExternalInput")
    smat = nc.dram_tensor("smat", [P, TC * P], bf16, kind="ExternalInput")
    out = nc.dram_tensor("out", [SHARD, F_OUT], f32, kind="ExternalOutput")

    T1_local = nc.dram_tensor("T1_local", [SHARD, F_HID], bf16)
    T1_full = nc.dram_tensor("T1_full", [N, F_HID], bf16, addr_space="Shared")
    T2_local = nc.dram_tensor("T2_local", [SHARD, P], bf16)
    T2_full = nc.dram_tensor("T2_full", [N, P], bf16, addr_space="Shared")

    C_t = C_ts.sum(axis=1)          # chunks per tile
    CMAX = int(C_ts.max())
    CTM = int(C_t.max())

    with tile.TileContext(nc) as tc:
        with (
            tc.tile_pool(name="const", bufs=1) as cpool,
            tc.tile_pool(name="xa", bufs=3) as xapool,
            tc.tile_pool(name="s1o", bufs=3) as s1pool,
            tc.tile_pool(name="g", bufs=10) as gpool,
            tc.tile_pool(name="sm", bufs=3) as smpool,
            tc.tile_pool(name="meta", bufs=3) as mpool,
            tc.tile_pool(name="ep", bufs=4) as eppool,
            tc.tile_pool(name="pa", bufs=2, space="PSUM") as papool,
            tc.tile_pool(name="ph", bufs=2, space="PSUM") as phpool,
            tc.tile_pool(name="ps", bufs=2, space="PSUM") as pspool,
        ):
            nc.gpsimd.load_library(mlp)

            # ---- resident constants ----
            W1_sb = cpool.tile([P, 4, F_HID], bf16)
            nc.sync.dma_start(
                W1_sb[:], W1b.ap().rearrange("(kk p) f -> p kk f", p=P))
            W2_sb = cpool.tile([P, F_OUT], bf16)
            nc.sync.dma_start(W2_sb[:], W2b.ap())
            b1_sb = cpool.tile([P, 1], f32)
            nc.sync.dma_start(b1_sb[:], b1c.ap())
            b2_sb = cpool.tile([P, F_OUT], f32)
            nc.sync.dma_start(b2_sb[:], b2bc.ap())

            # ---- phase A: support1 = x_k @ W1 -> T1_local (bf16) ----
            for m in range(NT):
                m0 = m * P
                mw = min(P, SHARD - m0)
                xa = xapool.tile([P, 4, P], bf16)
                nc.sync.dma_start(
                    xa[:, :, :mw],
                    xT.ap()[:, m0:m0 + mw].rearrange("(kk p) m -> p kk m", p=P))
                ps = papool.tile([P, F_HID], f32, space="PSUM")
                for kk in range(4):
                    nc.tensor.matmul(ps[:mw, :], xa[:, kk, :mw], W1_sb[:, kk, :],
                                     start=(kk == 0), stop=(kk == 3))
                s1 = s1pool.tile([P, F_HID], bf16)
                nc.scalar.activation(s1[:mw, :], ps[:mw, :],
                                     mybir.ActivationFunctionType.Copy)
                nc.sync.dma_start(T1_local.ap()[m0:m0 + mw, :], s1[:mw, :])

            # ---- AllGather T1 ----
            nc.gpsimd.collective_compute(
                "AllGather", mybir.AluOpType.bypass,
                replica_groups=[list(range(NC))],
                ins=[T1_local.ap().opt()],
                outs=[T1_full.ap().opt()],
            )

            # ---- phase B: SpMM1 + Relu + @W2 -> T2_local ----
            for t in range(NT):
                t0 = t * P
                tw = min(P, SHARD - t0)
                ct = int(C_t[t])
                coff = int(off_ts[t, 0])
                ix = mpool.tile([P, 8 * CMAX * NSEG], i16, tag="ix")
                nc.sync.dma_start(ix[:, :8 * ct],
                                  idx16.ap()[:, 8 * coff:8 * (coff + ct)])
                sm = smpool.tile([P, CTM, P], bf16, tag="sm")
                nc.sync.dma_start(
                    sm[:, :ct, :], smat.ap()[:, coff * P:(coff + ct) * P])

                ph = phpool.tile([P, P], f32, space="PSUM")
                ci = 0
                for s in range(NSEG):
                    cs = int(C_ts[t, s])
                    if cs == 0:
                        continue
                    local_off = int(off_ts[t, s]) - coff
                    g = gpool.tile([P, CMAX, P], bf16, tag="g")
                    nc.gpsimd.dma_gather(
                        g[:, :cs, :],
                        T1_full.ap()[s * SEG:(s + 1) * SEG, :],
                        ix[:, 8 * local_off:8 * (local_off + cs)],
                        cs * P, cs * P, F_HID,
                        single_packet=False, queue_num=s,
                    )
                    for c in range(cs):
                        nc.tensor.matmul(ph[:], g[:, c, :],
                                         sm[:, local_off + c, :],
                                         start=(ci == 0), stop=(ci == ct - 1))
                        ci += 1
                # h^T = relu(ph + b1) ; support2 = h @ W2
                hT = eppool.tile([P, P], bf16, tag="hT")
                nc.scalar.activation(hT[:], ph[:],
                                     mybir.ActivationFunctionType.Relu,
                                     bias=b1_sb[:])
                ps2 = pspool.tile([P, F_OUT], f32, space="PSUM")
                nc.tensor.matmul(ps2[:], hT[:], W2_sb[:], start=True, stop=True)
                s2 = eppool.tile([P, F_OUT], bf16, tag="s2")
                nc.vector.tensor_copy(s2[:], ps2[:])
                nc.sync.dma_start(T2_local.ap()[t0:t0 + tw, :F_OUT], s2[:tw, :])

            # ---- AllGather T2 ----
            nc.gpsimd.collective_compute(
                "AllGather", mybir.AluOpType.bypass,
                replica_groups=[list(range(NC))],
                ins=[T2_local.ap().opt()],
                outs=[T2_full.ap().opt()],
            )

            # ---- phase D: SpMM2 + bias + log_softmax -> out ----
            for t in range(NT):
                t0 = t * P
                tw = min(P, SHARD - t0)
                ct = int(C_t[t])
                coff = int(off_ts[t, 0])
                ix = mpool.tile([P, 8 * CMAX * NSEG], i16, tag="ix")
                nc.sync.dma_start(ix[:, :8 * ct],
                                  idx16.ap()[:, 8 * coff:8 * (coff + ct)])
                sm = smpool.tile([P, CTM, P], bf16, tag="sm")
                nc.sync.dma_start(
                    sm[:, :ct, :], smat.ap()[:, coff * P:(coff + ct) * P])

                pl = pspool.tile([P, F_OUT], f32, space="PSUM", tag="pl")
                ci = 0
                for s in range(NSEG):
                    cs = int(C_ts[t, s])
                    if cs == 0:
                        continue
                    local_off = int(off_ts[t, s]) - coff
                    g = gpool.tile([P, CMAX, P], bf16, tag="g")
                    nc.gpsimd.dma_gather(
                        g[:, :cs, :],
                        T2_full.ap()[s * SEG:(s + 1) * SEG, :],
                        ix[:, 8 * local_off:8 * (local_off + cs)],
                        cs * P, cs * P, P,
                        single_packet=False, queue_num=s,
                    )
                    for c in range(cs):
                        nc.tensor.matmul(pl[:], sm[:, local_off + c, :],
                                         g[:, c, :F_OUT],
                                         start=(ci == 0), stop=(ci == ct - 1))
                        ci += 1
                # logits = pl + b2 ; out = log_softmax(logits)
                lg = eppool.tile([P, F_OUT], f32, tag="lg")
                nc.vector.tensor_add(lg[:], pl[:], b2_sb[:])
                mx = eppool.tile([P, 1], f32, tag="mx")
                nc.vector.reduce_max(mx[:], lg[:], axis=mybir.AxisListType.X)
                nmx = eppool.tile([P, 1], f32, tag="nmx")
                nc.vector.tensor_scalar_mul(nmx[:], mx[:], -1.0)
                ex = eppool.tile([P, F_OUT], f32, tag="ex")
                nc.scalar.activation(ex[:], lg[:],
                                     mybir.ActivationFunctionType.Exp,
                                     bias=nmx[:])
                sme = eppool.tile([P, 1], f32, tag="sme")
                nc.vector.reduce_sum(sme[:], ex[:], axis=mybir.AxisListType.X)
                lns = eppool.tile([P, 1], f32, tag="lns")
                nc.scalar.activation(lns[:], sme[:],
                                     mybir.ActivationFunctionType.Ln)
                mls = eppool.tile([P, 1], f32, tag="mls")
                nc.vector.tensor_add(mls[:], mx[:], lns[:])
                oo = eppool.tile([P, F_OUT], f32, tag="oo")
                nc.vector.tensor_scalar(
                    oo[:], lg[:], mls[:], None,
                    op0=mybir.AluOpType.subtract)
                nc.sync.dma_start(out.ap()[t0:t0 + tw, :], oo[:tw, :])

    nc.compile()
    return nc


def _prepare(x, edge_row, edge_col, edge_val, W1, b1, W2, b2):
    C_ts, off_ts, TC, idx16, smats = _preprocess(
        np.asarray(edge_row), np.asarray(edge_col), np.asarray(edge_val))
    nc = _build_program(C_ts, off_ts, TC)

    x = np.asarray(x, np.float32)
    W1 = np.asarray(W1, np.float32)
    W2 = np.asarray(W2, np.float32)
    b1 = np.asarray(b1, np.float32)
    b2 = np.asarray(b2, np.float32)

    b1_np = b1.reshape(F_HID, 1).astype(np.float32)
    b2_np = np.broadcast_to(b2[None, :], (P, F_OUT)).copy().astype(np.float32)
    W1_np = W1.astype(BF16)
    W2_np = W2.astype(BF16)

    in_maps = []
    for c in range(NC):
        xk = x[c * SHARD:(c + 1) * SHARD]
        in_maps.append({
            "xT": np.ascontiguousarray(xk.T).astype(BF16),
            "W1b": W1_np, "W2b": W2_np,
            "b1c": b1_np, "b2bc": b2_np,
            "idx16": idx16[c], "smat": smats[c],
        })

    return nc, in_maps


def kernel(x, edge_row, edge_col, edge_val, W1, b1, W2, b2):
    nc, in_maps = _prepare(x, edge_row, edge_col, edge_val, W1, b1, W2, b2)
    res = run_bass_kernel_spmd(nc, in_maps, core_ids=list(range(NC)),
                               trace=False)
    return np.concatenate([res.results[c]["out"] for c in range(NC)], axis=0)
